# revision 9
# baseline (speedup 1.0000x reference)
"""GreedySampler Trainium2 kernel (fp8 tensor-parallel over vocab).

Strategy (per sharding hint): shard embd_weight along vocab across the 8
NeuronCores. Host gathers the 32 last-token hidden states (cumsum of
prefill_lens), scales + casts both operands to fp8-e4m3; each core
computes a [32, V_CORE] logits slab with DoubleRow fp8 PE matmuls
(256-deep contraction per instruction, 2x stream rate) and reduces each
vocab block to top-8 values + indices with the DVE Max/MaxIndex
instructions. Host combines the per-block top-8 candidates from all
cores and rescores the best ~48 per sequence against the original fp32
weights in fp64, so the returned argmax is exact as long as the true
argmax lands anywhere in the device's fp8 top-candidate set (measured
margin: the true argmax is rank 0 of the fp8 ordering for all 32
sequences on the problem's fixed seed; fp8 logit noise std ~0.048 vs
median top-2 gap ~0.26).

Roofline: fp8 halves HBM weight traffic vs fp16 (25.7 MB/core,
~72 us at ~360 GB/s) and DoubleRow keeps the PE well under that
(~37 us), so the kernel is DMA-bound. Full vocab blocks are DMA'd in
pairs (32 KB per partition line) to halve the per-transfer fixed
overhead on the saturated DMA path.
"""

import numpy as np
import ml_dtypes

NUM_SEQS = 32
D_MODEL = 4096
VOCAB = 50257
N_CORES = 8
BS = 512                    # vocab block (one PSUM bank of fp32)
NBF = 12                    # full 512-wide blocks per core
NPAIR = NBF // 2            # block pairs per weight DMA
BST = 139                   # tail block width
NB = NBF + 1                # 13 blocks per core
V_CORE = NBF * BS + BST     # 6283; 8*6283 = 50264 >= 50257
KT = D_MODEL // 128         # 32 k-tiles
W_SCALE = np.float32(256.0)
H_SCALE = np.float32(16.0)

HT_SHAPE = (128, KT, NUM_SEQS)
NP_F8 = ml_dtypes.float8_e4m3

_CACHE: dict = {}


def _build(loop_iters=None, bench_internal=False, max_unroll=4,
           dual_queue=False):
    """Build the SPMD program. With loop_iters=R, wrap the whole pass in a
    hardware loop (benchmarking variant; same per-pass instruction stream;
    max_unroll=0 instead repeats the pass as straight-line code for the
    timeline simulator). bench_internal=True makes the weights Internal
    DRAM (uninitialized) so benchmark calls only transfer the tiny ht
    input; the kernel's HBM traffic is unchanged."""
    import concourse.tile as tile
    from concourse import bacc, mybir

    nc = bacc.Bacc("TRN2", target_bir_lowering=False, debug=False,
                   num_devices=N_CORES)
    f8 = mybir.dt.float8e4
    f32 = mybir.dt.float32
    u32 = mybir.dt.uint32
    dr = mybir.MatmulPerfMode.DoubleRow

    wkind = "Internal" if bench_internal else "ExternalInput"
    ht = nc.dram_tensor("ht", list(HT_SHAPE), f8, kind="ExternalInput")
    wt = nc.dram_tensor("wt", [NPAIR, 128, 2, KT, BS], f8, kind=wkind)
    wtt = nc.dram_tensor("wtt", [128, KT, BST], f8, kind=wkind)
    out_v = nc.dram_tensor("out_v", [NUM_SEQS, NB * 8], f32,
                           kind="ExternalOutput")
    out_i = nc.dram_tensor("out_i", [NUM_SEQS, NB * 8], u32,
                           kind="ExternalOutput")

    with tile.TileContext(nc) as tc:
        with (
            tc.tile_pool(name="htp", bufs=1) as htp,
            tc.tile_pool(name="wp", bufs=3) as wp,
            tc.tile_pool(name="wtp", bufs=2) as wtp,
            tc.tile_pool(name="lgp", bufs=3) as lgp,
            tc.tile_pool(name="smp", bufs=2) as smp,
            tc.tile_pool(name="psp", bufs=4, space="PSUM") as psp,
        ):
            ht_t = htp.tile(list(HT_SHAPE), f8)
            nc.sync.dma_start(ht_t[:], ht[:])

            def reduce_block(wslice, b, bs, mxall, ixall):
                """Matmul one vocab block and fold it to top-8 (vals+idx)."""
                ps = psp.tile([NUM_SEQS, bs], f32, tag="ps")
                for k in range(0, KT, 2):
                    nc.tensor.matmul(
                        ps[:],
                        ht_t[:, k:k + 2, :],
                        wslice[:, k:k + 2, :],
                        start=(k == 0),
                        stop=(k == KT - 2),
                        perf_mode=dr,
                    )
                lg = lgp.tile([NUM_SEQS, bs], f32, tag="lg")
                nc.vector.tensor_copy(lg[:], ps[:])
                nc.vector.max(mxall[:, b * 8:(b + 1) * 8], lg[:])
                nc.vector.max_index(ixall[:, b * 8:(b + 1) * 8],
                                    mxall[:, b * 8:(b + 1) * 8], lg[:])

            def one_pass(_iv=None, unroll=None):
                mxall = smp.tile([NUM_SEQS, NB * 8], f32)
                ixall = smp.tile([NUM_SEQS, NB * 8], u32)

                for pair in range(NPAIR):
                    wt_t = wp.tile([128, 2, KT, BS], f8, tag="wt")
                    eng = (nc.scalar if dual_queue and pair % 2 else nc.sync)
                    eng.dma_start(wt_t[:], wt[pair])
                    for j in range(2):
                        reduce_block(wt_t[:, j], 2 * pair + j, BS,
                                     mxall, ixall)

                wtt_t = wtp.tile([128, KT, BST], f8, tag="wtt")
                nc.sync.dma_start(wtt_t[:], wtt[:])
                reduce_block(wtt_t, NBF, BST, mxall, ixall)

                nc.sync.dma_start(out_v[:], mxall[:])
                nc.sync.dma_start(out_i[:], ixall[:])

            if loop_iters is None:
                one_pass()
            elif max_unroll == 0:               # static straight-line repeat
                for _ in range(loop_iters):
                    one_pass()
            else:
                tc.For_i_unrolled(0, loop_iters, 1, one_pass,
                                  max_unroll=max_unroll)

    nc.compile()
    return nc


def _get_nc():
    if "nc" not in _CACHE:
        _CACHE["nc"] = _build()
    return _CACHE["nc"]


def _prep_inputs(hidden_states, embd_weight, prefill_lens):
    idx = np.cumsum(prefill_lens.astype(np.int64)) - 1
    last_h = np.ascontiguousarray(hidden_states[idx])       # [32, 4096] f32

    # [128, KT, 32] fp8: line p holds, for each k-tile, the 32 seq values
    ht_part = np.ascontiguousarray(
        (last_h.T * H_SCALE).reshape(KT, 128, NUM_SEQS).transpose(1, 0, 2)
    ).astype(NP_F8)

    in_maps = []
    for c in range(N_CORES):
        lo = c * V_CORE
        hi = min((c + 1) * V_CORE, VOCAB)
        slab = (embd_weight[lo:hi] * W_SCALE).astype(NP_F8)  # [<=6283, 4096]
        if hi - lo < V_CORE:                                # pad with last row
            pad = np.broadcast_to(slab[-1], (V_CORE - (hi - lo), D_MODEL))
            slab = np.concatenate([slab, pad], axis=0)
        # [V_CORE, D] -> block pairs [NPAIR, 128, 2, KT, BS];
        # line p = [block 2i: (kt, col), block 2i+1: (kt, col)]
        main = slab[:NBF * BS]
        wt_core = np.ascontiguousarray(
            main.reshape(NPAIR, 2, BS, KT, 128).transpose(0, 4, 1, 3, 2))
        tail = slab[NBF * BS:]
        wtt_core = np.ascontiguousarray(
            tail.reshape(BST, KT, 128).transpose(2, 1, 0))
        in_maps.append({"ht": ht_part, "wt": wt_core, "wtt": wtt_core})
    return in_maps, last_h


def _combine(results, last_h, embd_weight, n_rescore=48):
    """Merge per-core/block top-8 candidates; rescore the best n_rescore
    per sequence against the fp32 weights in fp64 so the argmax is exact."""
    vals = np.stack([results[c]["out_v"] for c in range(N_CORES)])
    idxs = np.stack([results[c]["out_i"] for c in range(N_CORES)])
    # [c, s, NB*8] -> candidate global vocab ids
    base = (np.arange(N_CORES)[:, None] * V_CORE
            + np.arange(NB)[None, :] * BS)                  # [c, b]
    gid = (idxs.reshape(N_CORES, NUM_SEQS, NB, 8)
           + base[:, None, :, None]).astype(np.int64)       # [c, s, b, 8]
    gid = np.minimum(gid, VOCAB - 1)
    cand_v = vals.transpose(1, 0, 2).reshape(NUM_SEQS, -1)  # [s, c*NB*8]
    cand_g = gid.transpose(1, 0, 2, 3).reshape(NUM_SEQS, -1)

    out = np.empty(NUM_SEQS, np.int32)
    h64 = last_h.astype(np.float64)
    for s in range(NUM_SEQS):
        top = np.argsort(-cand_v[s], kind="stable")[:n_rescore]
        g = np.unique(cand_g[s, top])                       # ascending ids
        scores = embd_weight[g].astype(np.float64) @ h64[s]
        out[s] = g[np.argmax(scores)]                       # tie -> lowest id
    return out


def _run_checked(nc, in_maps, n_attempts=4):
    """Run the SPMD kernel; retry if any core returned NaN block maxima
    (observed transiently on the very first NEFF execution in a process)."""
    from concourse.bass_utils import run_bass_kernel_spmd

    last = None
    for _ in range(n_attempts):
        res = run_bass_kernel_spmd(nc, in_maps, list(range(N_CORES)))
        last = res.results
        ok = all(
            np.isfinite(last[c]["out_v"]).all()
            and (last[c]["out_i"][:, :NBF * 8] < BS).all()
            and (last[c]["out_i"][:, NBF * 8:] < BST).all()
            for c in range(N_CORES)
        )
        if ok:
            return last
    return last


def kernel(hidden_states, embd_weight, prefill_lens):
    nc = _get_nc()
    hidden_states = np.asarray(hidden_states)
    embd_weight = np.asarray(embd_weight)
    in_maps, last_h = _prep_inputs(hidden_states, embd_weight,
                                   np.asarray(prefill_lens))
    results = _run_checked(nc, in_maps)
    return _combine(results, last_h, embd_weight)
